# revision 1
# baseline (speedup 1.0000x reference)
"""Trainium2 Bass kernel for the additive-attention module.

Reference math (single device):
    enc    = einsum('sbh,kh->sbk', encoder_output, We) + be     # [S,B,K]
    hid    = hidden @ Wh.T + bh                                 # [B,K]
    energy = sigmoid(enc + hid[None]) @ Wv.T + bv               # [S,B,1]
    attn   = softmax(energy, axis=0)                            # over S
    out    = attn.transpose(1, 2, 0)                            # [B,1,S]

Device strategy (8 NeuronCores, data-parallel over batch):
  * Each core gets 8 of the 64 batches; weights replicated.
  * encoder_output is re-laid-out on the host to [H, B_core, S] so the
    contraction dim H lands on SBUF partitions (b-major so the per-batch
    hid term is a per-partition ACT bias).  The fp32 HBM data is cast to
    bf16 in-flight by the SWDGE DMA.
  * sigmoid(x) = (1 + tanh(x/2))/2, and softmax is invariant to the
    affine constants, so on device we compute
        E[s,b] = sum_k Wv[k] * tanh(0.5*enc_raw + hidb)
    (hidb = 0.5*(hidden @ Wh.T + bh + be), computed on host - 17 MFLOP)
    and finish with softmax(0.5 * E).  tanh shares the ACT table set
    with exp, so there is no table thrashing.
  * The Wv reduction over k rides the PE as matmuls with a zero-padded
    [128,128] stationary operand (column 0 = Wv chunk) so the weight
    load takes the FWL path and hides under the previous matmul.
  * enc matmuls run as fp8e4 DoubleRow (2 MACs/cell/cycle); We is
    host-scaled by 32 for fp8 range, compensated in the ACT scale.
    Softmax needs no max-subtraction: |0.5*E| <= 0.5*sum|Wv| ~ 5.7.
"""

import os
import numpy as np

import concourse.bass as bass
import concourse.mybir as mybir
import concourse.tile as tile
from concourse import bacc
from concourse.bass_utils import run_bass_kernel_spmd

S_TOT = 4096
B_TOT = 64
H = 512
N_CORES = 8
BPC = B_TOT // N_CORES  # batches per core
P = 128
KC = H // P  # 4 contraction / output chunks
SH = 1024    # s-chunk processed per tanh tile
NMM = 512    # matmul moving free dim

F32 = mybir.dt.float32
BF16 = mybir.dt.bfloat16
F8 = mybir.dt.float8e4
WE_SCALE = 32.0

# Results of the most recent device run (for the local test harness only).
LAST_RESULTS = None

_BUILD_CACHE = {}


def _build(s_tot=S_TOT, bpc=BPC, n_cores=N_CORES):
    key = (s_tot, bpc, n_cores)
    if key in _BUILD_CACHE:
        return _BUILD_CACHE[key]

    nc = bacc.Bacc(
        "TRN2", target_bir_lowering=False, debug=False, num_devices=n_cores
    )
    eoT = nc.dram_tensor("eoT", [H, bpc, s_tot], F32, kind="ExternalInput")
    WeT = nc.dram_tensor("WeT", [P, KC, H], F8, kind="ExternalInput")
    hidb = nc.dram_tensor("hidb", [P, KC, bpc], F32, kind="ExternalInput")
    Wvp = nc.dram_tensor("Wvp", [P, KC * P], BF16, kind="ExternalInput")
    out = nc.dram_tensor("out", [bpc, s_tot], F32, kind="ExternalOutput")

    sh = min(SH, s_tot)
    nns = sh // NMM
    Tanh = mybir.ActivationFunctionType.Tanh
    Exp = mybir.ActivationFunctionType.Exp

    with tile.TileContext(nc) as tc:
        with (
            tc.tile_pool(name="weights", bufs=1) as wpool,
            tc.tile_pool(name="ebuf", bufs=6) as epool,
            tc.tile_pool(name="sig", bufs=4) as sigpool,
            tc.tile_pool(name="estage", bufs=2) as stpool,
            tc.tile_pool(name="small", bufs=1) as spool,
            tc.tile_pool(name="enc", bufs=3, space="PSUM") as encpool,
            tc.tile_pool(name="epsum", bufs=2, space="PSUM") as enpool,
        ):
            WeT_sb = wpool.tile([P, KC, H], F8, tag="WeT")
            nc.sync.dma_start(WeT_sb[:], WeT.ap())
            hidb_sb = wpool.tile([P, KC, bpc], F32, tag="hidb")
            nc.sync.dma_start(hidb_sb[:], hidb.ap())
            Wv_sb = wpool.tile([P, KC * P], BF16, tag="Wvp")
            nc.sync.dma_start(Wv_sb[:], Wvp.ap())

            energy_sb = spool.tile([bpc, s_tot], F32, tag="energy")
            pexp = spool.tile([bpc, s_tot], F32, tag="pexp")
            eoT_r = eoT.ap().rearrange("(c p) b s -> p c b s", p=P)

            s_blk = min(SH, s_tot)
            nblk = s_tot // s_blk
            # per-block softmax partials (online softmax over s)
            sloc = spool.tile([bpc, nblk], F32, tag="sloc")   # rowsum(exp) per blk
            for blk in range(nblk):
                sb0 = blk * s_blk
                for b in range(bpc):
                    ebuf = epool.tile([P, KC, s_blk], F8, tag="ebuf")
                    nc.gpsimd.dma_start(
                        ebuf[:], eoT_r[:, :, b, sb0:sb0 + s_blk]
                    )
                    stage = stpool.tile([1, s_blk], F32, tag="estage")
                    eps = [
                        enpool.tile([P, NMM], F32, tag="epsum", name=f"eps{ns}")
                        for ns in range(nns)
                    ]
                    for kc in range(KC):
                        enc = encpool.tile([P, sh], F32, tag="enc")
                        for ns in range(nns):
                            s0 = ns * NMM
                            for hc in range(0, KC, 2):
                                nc.tensor.matmul(
                                    enc[:, s0:s0 + NMM],
                                    WeT_sb[:, hc:hc + 2, kc * P:(kc + 1) * P],
                                    ebuf[:, hc:hc + 2, s0:s0 + NMM],
                                    start=(hc == 0),
                                    stop=(hc == KC - 2),
                                    perf_mode=mybir.MatmulPerfMode.DoubleRow,
                                )
                        sig = sigpool.tile([P, sh], BF16, tag="sig")
                        nc.scalar.activation(
                            sig[:], enc[:], Tanh,
                            scale=0.5 / WE_SCALE, bias=hidb_sb[:, kc, b:b + 1],
                        )
                        for ns in range(nns):
                            nc.tensor.matmul(
                                eps[ns][:, :],
                                Wv_sb[:, kc * P:(kc + 1) * P],
                                sig[:, ns * NMM:(ns + 1) * NMM],
                                start=(kc == 0),
                                stop=(kc == KC - 1),
                            )
                    for ns in range(nns):
                        nc.vector.tensor_copy(
                            stage[0:1, ns * NMM:(ns + 1) * NMM], eps[ns][0:1, :]
                        )
                    nc.sync.dma_start(
                        energy_sb[b:b + 1, sb0:sb0 + s_blk], stage[:]
                    )
                # partial softmax stats for this block (runs under next
                # block).  |0.5*E| <= 0.5*sum|Wv| ~ 5.7, so exp cannot
                # overflow fp32 and no max-subtraction is needed.
                eblk = energy_sb[:, sb0:sb0 + s_blk]
                nc.scalar.activation(
                    pexp[:, sb0:sb0 + s_blk], eblk, Exp, scale=0.5,
                )
                nc.vector.tensor_reduce(
                    out=sloc[:, blk:blk + 1], in_=pexp[:, sb0:sb0 + s_blk],
                    axis=mybir.AxisListType.X, op=mybir.AluOpType.add,
                )

            # attn = pexp / rowsum(pexp)
            stot = spool.tile([bpc, 1], F32, tag="stot")
            nc.vector.tensor_reduce(
                out=stot[:], in_=sloc[:],
                axis=mybir.AxisListType.X, op=mybir.AluOpType.add,
            )
            rec = spool.tile([bpc, 1], F32, tag="rec")
            nc.vector.reciprocal(rec[:], stot[:])
            for blk in range(nblk):
                sb0 = blk * s_blk
                nc.vector.tensor_scalar_mul(
                    pexp[:, sb0:sb0 + s_blk], pexp[:, sb0:sb0 + s_blk],
                    rec[:],
                )
            nc.sync.dma_start(out.ap()[:, :], pexp[:])

    nc.compile()
    _BUILD_CACHE[key] = nc
    return nc


def make_in_maps(hidden, encoder_output, We, be, Wh, bh, Wv):
    """Host-side sharding/layout prep. Returns per-core input dicts."""
    import ml_dtypes
    eo = np.ascontiguousarray(np.asarray(encoder_output, dtype=np.float32))
    hidden = np.asarray(hidden, dtype=np.float32)
    WeT = np.ascontiguousarray(
        (np.asarray(We, np.float32).T * WE_SCALE)
        .reshape(KC, P, H).transpose(1, 0, 2)
    ).astype(ml_dtypes.float8_e4m3fn)  # [P, KC(hc), H(k)]

    # hidb = 0.5 * (hidden @ Wh.T + bh + be), laid out [P, KC, B]
    hid_all = 0.5 * (
        hidden @ np.asarray(Wh, np.float32).T
        + np.asarray(bh, np.float32) + np.asarray(be, np.float32)
    )  # [B_TOT, H]
    # Wv padded stationary operand: [P, KC*P], column 0 of each kc block
    Wvp = np.zeros((P, KC * P), np.float32)
    wv = np.asarray(Wv, np.float32).reshape(-1)  # [H]
    for kc in range(KC):
        Wvp[:, kc * P] = wv[kc * P:(kc + 1) * P]
    Wvp = Wvp.astype(ml_dtypes.bfloat16)

    in_maps = []
    for c in range(N_CORES):
        b0 = c * BPC
        eoT_c = np.ascontiguousarray(
            eo[:, b0:b0 + BPC, :].transpose(2, 1, 0)
        )  # [H, BPC, S]
        hidb_c = np.ascontiguousarray(
            hid_all[b0:b0 + BPC].T.reshape(KC, P, BPC).transpose(1, 0, 2)
        )  # [P, KC, BPC]
        in_maps.append({
            "eoT": eoT_c,
            "WeT": WeT,
            "hidb": hidb_c,
            "Wvp": Wvp,
        })
    return in_maps


def kernel(hidden, encoder_output, each_size=None, We=None, be=None,
           Wh=None, bh=None, Wv=None, bv=None):
    global LAST_RESULTS
    nc = _build()
    in_maps = make_in_maps(hidden, encoder_output, We, be, Wh, bh, Wv)
    res = run_bass_kernel_spmd(
        nc, in_maps, list(range(N_CORES)),
        trace=bool(os.environ.get("BASS_TRACE")),
    )
    LAST_RESULTS = res
    attn = np.concatenate([res.results[c]["out"] for c in range(N_CORES)], axis=0)
    return np.ascontiguousarray(attn.reshape(B_TOT, 1, S_TOT).astype(np.float32))



# revision 5
# speedup vs baseline: 1.0451x; 1.0451x over previous
"""Trainium2 Bass kernel for the additive-attention module.

Reference math (single device):
    enc    = einsum('sbh,kh->sbk', encoder_output, We) + be     # [S,B,K]
    hid    = hidden @ Wh.T + bh                                 # [B,K]
    energy = sigmoid(enc + hid[None]) @ Wv.T + bv               # [S,B,1]
    attn   = softmax(energy, axis=0)                            # over S
    out    = attn.transpose(1, 2, 0)                            # [B,1,S]

Device strategy (8 NeuronCores, data-parallel over batch):
  * Each core gets 8 of the 64 batches; weights replicated.
  * encoder_output is cast to fp8e4m3 ON THE HOST and laid out
    [P, KC(hc), B_core, S] so HBM traffic is 16 MiB/core (fp8) instead
    of 64 MiB (fp32).  The whole per-core slice stays SBUF-resident.
  * sigmoid(x) = (1 + tanh(x/2))/2, softmax is invariant to affine
    constants, so the device computes
        E[s,b] = sum_k Wv[k] * tanh(0.5*enc_raw/WE_SCALE + hidb)
    (hidb = 0.5*(hidden @ Wh.T + bh + be), computed on host)
    and finishes with softmax(0.5 * E).  tanh shares the ACT table set
    with exp, so there is no table thrashing.
  * enc matmuls run as fp8e4 DoubleRow (2 MACs/cell/cycle); We is
    host-scaled by 32 for fp8 range, compensated in the ACT scale.
  * The Wv-weighted reduction over k runs as a fused DVE chain
    (scalar_tensor_tensor: acc = sig_kc*wv_kc + acc over the 4 kc
    chunks, bf16 at 2x mode) followed by a trivial [128,1] ones-vector
    matmul on the PE for the cross-partition sum.  This takes ~55us of
    matmul work off the Tensor engine vs. pumping raw sig through it.
  * Softmax needs no max-subtraction: |0.5*E| <= 0.5*sum|Wv| ~ 5.7.
"""

import os
import numpy as np

import concourse.bass as bass
import concourse.mybir as mybir
import concourse.tile as tile
from concourse import bacc
from concourse.bass_utils import run_bass_kernel_spmd

S_TOT = 4096
B_TOT = 64
H = 512
N_CORES = 8
BPC = B_TOT // N_CORES  # batches per core
P = 128
KC = H // P   # 4 contraction / output chunks
SBLK = 1024   # s-chunk per (b, blk) group
NMM = 512     # matmul moving free dim (one PSUM bank)

F32 = mybir.dt.float32
BF16 = mybir.dt.bfloat16
F8 = mybir.dt.float8e4
WE_SCALE = 32.0

# Results of the most recent device run (for the local test harness only).
LAST_RESULTS = None

_BUILD_CACHE = {}


def _build(s_tot=S_TOT, bpc=BPC, n_cores=N_CORES):
    key = (s_tot, bpc, n_cores)
    if key in _BUILD_CACHE:
        return _BUILD_CACHE[key]

    nc = bacc.Bacc(
        "TRN2", target_bir_lowering=False, debug=False, num_devices=n_cores
    )
    eoT = nc.dram_tensor("eoT", [P, KC, bpc, s_tot], F8, kind="ExternalInput")
    WeT = nc.dram_tensor("WeT", [P, KC, H], F8, kind="ExternalInput")
    hidb = nc.dram_tensor("hidb", [P, KC, bpc], F32, kind="ExternalInput")
    wvp = nc.dram_tensor("wvp", [P, KC], F32, kind="ExternalInput")
    ones = nc.dram_tensor("ones", [P, 1], BF16, kind="ExternalInput")
    out = nc.dram_tensor("out", [bpc, s_tot], F32, kind="ExternalOutput")

    nblk = s_tot // SBLK
    nns = SBLK // NMM
    Tanh = mybir.ActivationFunctionType.Tanh
    Exp = mybir.ActivationFunctionType.Exp
    Mult = mybir.AluOpType.mult
    Add = mybir.AluOpType.add
    DR = mybir.MatmulPerfMode.DoubleRow

    with tile.TileContext(nc) as tc:
        with (
            tc.tile_pool(name="weights", bufs=1) as wpool,
            tc.tile_pool(name="eo", bufs=1) as eopool,
            tc.tile_pool(name="sig", bufs=3) as sigpool,
            tc.tile_pool(name="acc", bufs=2) as accpool,
            tc.tile_pool(name="stage", bufs=2) as stpool,
            tc.tile_pool(name="enc", bufs=3, space="PSUM") as encpool,
            tc.tile_pool(name="eps", bufs=1, space="PSUM") as epspool,
        ):
            WeT_sb = wpool.tile([P, KC, H], F8, tag="WeT")
            nc.sync.dma_start(WeT_sb[:], WeT.ap())
            hidb_sb = wpool.tile([P, KC, bpc], F32, tag="hidb")
            nc.sync.dma_start(hidb_sb[:], hidb.ap())
            wv_sb = wpool.tile([P, KC], F32, tag="wvp")
            nc.sync.dma_start(wv_sb[:], wvp.ap())
            ones_sb = wpool.tile([P, 1], BF16, tag="ones")
            nc.sync.dma_start(ones_sb[:], ones.ap())

            energy_sb = wpool.tile([bpc, s_tot], F32, tag="energy")
            pexp = wpool.tile([bpc, s_tot], F32, tag="pexp")
            sloc = wpool.tile([bpc, nblk], F32, tag="sloc")

            # Stream the whole per-core encoder slice into SBUF in
            # compute order; it stays resident (16 KiB/partition fp8).
            eo_t = {}
            for blk in range(nblk):
                for b in range(bpc):
                    t = eopool.tile([P, KC, SBLK], F8, tag=f"eo{blk}_{b}")
                    eo_t[(blk, b)] = t
                    nc.gpsimd.dma_start(
                        t[:], eoT.ap()[:, :, b, blk * SBLK:(blk + 1) * SBLK]
                    )

            for blk in range(nblk):
                sb0 = blk * SBLK
                for b in range(bpc):
                    eob = eo_t[(blk, b)]
                    sig = sigpool.tile([P, KC, SBLK], BF16, tag="sig")
                    for kc in range(KC):
                        enc = encpool.tile([P, SBLK], F32, tag="enc")
                        for hp in range(KC // 2):
                            for ns in range(nns):
                                nc.tensor.matmul(
                                    enc[:, ns * NMM:(ns + 1) * NMM],
                                    WeT_sb[:, 2 * hp:2 * hp + 2,
                                           kc * P:(kc + 1) * P],
                                    eob[:, 2 * hp:2 * hp + 2,
                                        ns * NMM:(ns + 1) * NMM],
                                    start=(hp == 0),
                                    stop=(hp == KC // 2 - 1),
                                    perf_mode=DR,
                                )
                        nc.scalar.activation(
                            sig[:, kc, :], enc[:], Tanh,
                            scale=0.5 / WE_SCALE, bias=hidb_sb[:, kc, b:b + 1],
                        )
                    # acc[p, s] = sum_kc wv[kc*P+p] * sig[p, kc, s]  (DVE)
                    acc = accpool.tile([P, SBLK], BF16, tag="acc")
                    nc.vector.tensor_scalar_mul(
                        acc[:], sig[:, 0, :], wv_sb[:, 0:1]
                    )
                    for kc in range(1, KC):
                        nc.vector.scalar_tensor_tensor(
                            acc[:], sig[:, kc, :], wv_sb[:, kc:kc + 1],
                            acc[:], op0=Mult, op1=Add,
                        )
                    # E[s] = sum_p acc[p, s] via ones-vector matmul
                    eps = epspool.tile([1, SBLK], F32, tag="eps")
                    for ns in range(nns):
                        nc.tensor.matmul(
                            eps[0:1, ns * NMM:(ns + 1) * NMM],
                            ones_sb[:, 0:1],
                            acc[:, ns * NMM:(ns + 1) * NMM],
                            start=True, stop=True,
                        )
                    stage = stpool.tile([1, SBLK], F32, tag="stage")
                    nc.vector.tensor_copy(stage[0:1, :], eps[0:1, :])
                    nc.sync.dma_start(
                        energy_sb[b:b + 1, sb0:sb0 + SBLK], stage[0:1, :]
                    )
                # softmax partials for this block (overlaps next block).
                # |0.5*E| <= 0.5*sum|Wv| ~ 5.7 so exp cannot overflow.
                nc.scalar.activation(
                    pexp[:, sb0:sb0 + SBLK], energy_sb[:, sb0:sb0 + SBLK],
                    Exp, scale=0.5, accum_out=sloc[:, blk:blk + 1],
                )

            # attn = pexp / rowsum(pexp)
            stot = wpool.tile([bpc, 1], F32, tag="stot")
            nc.vector.tensor_reduce(
                out=stot[:], in_=sloc[:],
                axis=mybir.AxisListType.X, op=Add,
            )
            rec = wpool.tile([bpc, 1], F32, tag="rec")
            nc.vector.reciprocal(rec[:], stot[:])
            for blk in range(nblk):
                sb0 = blk * SBLK
                nc.vector.tensor_scalar_mul(
                    pexp[:, sb0:sb0 + SBLK], pexp[:, sb0:sb0 + SBLK],
                    rec[:],
                )
                nc.sync.dma_start(
                    out.ap()[:, sb0:sb0 + SBLK], pexp[:, sb0:sb0 + SBLK]
                )

    nc.compile()
    _BUILD_CACHE[key] = nc
    return nc


def make_in_maps(hidden, encoder_output, We, be, Wh, bh, Wv):
    """Host-side sharding/layout prep. Returns per-core input dicts."""
    import ml_dtypes
    eo8 = np.asarray(encoder_output, dtype=np.float32).astype(
        ml_dtypes.float8_e4m3fn
    )  # [S, B, H]
    hidden = np.asarray(hidden, dtype=np.float32)
    WeT = np.ascontiguousarray(
        (np.asarray(We, np.float32).T * WE_SCALE)
        .reshape(KC, P, H).transpose(1, 0, 2)
    ).astype(ml_dtypes.float8_e4m3fn)  # [P, KC(hc), H(k)]

    # hidb = 0.5 * (hidden @ Wh.T + bh + be), laid out [P, KC, B]
    hid_all = 0.5 * (
        hidden @ np.asarray(Wh, np.float32).T
        + np.asarray(bh, np.float32) + np.asarray(be, np.float32)
    )  # [B_TOT, H]
    wv = np.asarray(Wv, np.float32).reshape(-1)           # [H]
    wvp = np.ascontiguousarray(wv.reshape(KC, P).T)        # [P, KC]
    ones = np.ones((P, 1), np.float32).astype(ml_dtypes.bfloat16)

    in_maps = []
    for c in range(N_CORES):
        b0 = c * BPC
        # [P, KC, BPC, S]: element (p, hc, b, s) = eo[s, b0+b, hc*128+p]
        eoT_c = np.ascontiguousarray(
            eo8[:, b0:b0 + BPC, :].transpose(2, 1, 0)
            .reshape(KC, P, BPC, S_TOT).transpose(1, 0, 2, 3)
        )
        hidb_c = np.ascontiguousarray(
            hid_all[b0:b0 + BPC].T.reshape(KC, P, BPC).transpose(1, 0, 2)
        )  # [P, KC, BPC]
        in_maps.append({
            "eoT": eoT_c,
            "WeT": WeT,
            "hidb": hidb_c,
            "wvp": wvp,
            "ones": ones,
        })
    return in_maps


def kernel(hidden, encoder_output, each_size=None, We=None, be=None,
           Wh=None, bh=None, Wv=None, bv=None):
    global LAST_RESULTS
    nc = _build()
    in_maps = make_in_maps(hidden, encoder_output, We, be, Wh, bh, Wv)
    res = run_bass_kernel_spmd(
        nc, in_maps, list(range(N_CORES)),
        trace=bool(os.environ.get("BASS_TRACE")),
    )
    LAST_RESULTS = res
    attn = np.concatenate([res.results[c]["out"] for c in range(N_CORES)], axis=0)
    return np.ascontiguousarray(attn.reshape(B_TOT, 1, S_TOT).astype(np.float32))


# revision 6
# speedup vs baseline: 1.5041x; 1.4392x over previous
"""Trainium2 Bass kernel for the additive-attention module.

Reference math (single device):
    enc    = einsum('sbh,kh->sbk', encoder_output, We) + be     # [S,B,K]
    hid    = hidden @ Wh.T + bh                                 # [B,K]
    energy = sigmoid(enc + hid[None]) @ Wv.T + bv               # [S,B,1]
    attn   = softmax(energy, axis=0)                            # over S
    out    = attn.transpose(1, 2, 0)                            # [B,1,S]

Device strategy (8 NeuronCores, data-parallel over batch):
  * Each core gets 8 of the 64 batches; weights replicated.
  * encoder_output is cast to fp8e4m3 ON THE HOST and laid out
    [P, KC(hc), B_core, S] so HBM traffic is 16 MiB/core (fp8) instead
    of 64 MiB (fp32).  The whole per-core slice stays SBUF-resident.
  * sigmoid(x) = (1 + tanh(x/2))/2, softmax is invariant to affine
    constants, so the device computes
        E[s,b] = sum_k Wv[k] * tanh(0.5*enc_raw/WE_SCALE + hidb)
    (hidb = 0.5*(hidden @ Wh.T + bh + be), computed on host)
    and finishes with softmax(0.5 * E).  tanh shares the ACT table set
    with exp, so there is no table thrashing.
  * enc matmuls run as fp8e4 DoubleRow (2 MACs/cell/cycle); We is
    host-scaled by 32 for fp8 range, compensated in the ACT scale.
  * The Wv-weighted reduction over k: DVE does the per-partition
    wv*sig multiply (tensor_scalar at 4x bf16) and the 4-chunk combine
    (tensor_add at 2x); the 128-partition sum rides the PE as a cheap
    one-hot-stationary matmul that deposits batch b's energies into
    row b of a single [BPC, SBLK] PSUM tile (accumulation over b adds
    exact zeros elsewhere).  The per-block exp then reads that PSUM
    tile directly on the scalar engine - no evacuation copies.
  * Softmax needs no max-subtraction: |0.5*E| <= 0.5*sum|Wv| ~ 5.7.
"""

import os
import numpy as np

import concourse.bass as bass
import concourse.mybir as mybir
import concourse.tile as tile
from concourse import bacc
from concourse.bass_utils import run_bass_kernel_spmd

S_TOT = 4096
B_TOT = 64
H = 512
N_CORES = 8
BPC = B_TOT // N_CORES  # batches per core
P = 128
KC = H // P   # 4 contraction / output chunks
SBLK = 1024   # s-chunk per (b, blk) group
NMM = 512     # matmul moving free dim (one PSUM bank)

F32 = mybir.dt.float32
BF16 = mybir.dt.bfloat16
F8 = mybir.dt.float8e4
WE_SCALE = 32.0

# Results of the most recent device run (for the local test harness only).
LAST_RESULTS = None

_BUILD_CACHE = {}


def _build(s_tot=S_TOT, bpc=BPC, n_cores=N_CORES):
    key = (s_tot, bpc, n_cores)
    if key in _BUILD_CACHE:
        return _BUILD_CACHE[key]

    nc = bacc.Bacc(
        "TRN2", target_bir_lowering=False, debug=False, num_devices=n_cores
    )
    eoT = nc.dram_tensor("eoT", [P, KC, bpc, s_tot], F8, kind="ExternalInput")
    WeT = nc.dram_tensor("WeT", [P, KC, H], F8, kind="ExternalInput")
    hidb = nc.dram_tensor("hidb", [P, KC, bpc], F32, kind="ExternalInput")
    wvp = nc.dram_tensor("wvp", [P, KC], F32, kind="ExternalInput")
    onehot = nc.dram_tensor("onehot", [P, bpc, bpc], BF16, kind="ExternalInput")
    out = nc.dram_tensor("out", [bpc, s_tot], F32, kind="ExternalOutput")

    nblk = s_tot // SBLK
    nns = SBLK // NMM
    Tanh = mybir.ActivationFunctionType.Tanh
    Exp = mybir.ActivationFunctionType.Exp
    Add = mybir.AluOpType.add
    DR = mybir.MatmulPerfMode.DoubleRow

    with tile.TileContext(nc) as tc:
        with (
            tc.tile_pool(name="weights", bufs=1) as wpool,
            tc.tile_pool(name="eo", bufs=1) as eopool,
            tc.tile_pool(name="sig", bufs=3) as sigpool,
            tc.tile_pool(name="acc", bufs=2) as accpool,
            tc.tile_pool(name="enc", bufs=3, space="PSUM") as encpool,
            tc.tile_pool(name="eps", bufs=1, space="PSUM") as epspool,
        ):
            WeT_sb = wpool.tile([P, KC, H], F8, tag="WeT")
            nc.sync.dma_start(WeT_sb[:], WeT.ap())
            hidb_sb = wpool.tile([P, KC, bpc], F32, tag="hidb")
            nc.sync.dma_start(hidb_sb[:], hidb.ap())
            wv_sb = wpool.tile([P, KC], F32, tag="wvp")
            nc.sync.dma_start(wv_sb[:], wvp.ap())
            oh_sb = wpool.tile([P, bpc, bpc], BF16, tag="onehot")
            nc.sync.dma_start(oh_sb[:], onehot.ap())

            pexp = wpool.tile([bpc, s_tot], F32, tag="pexp")
            sloc = wpool.tile([bpc, nblk], F32, tag="sloc")

            # Stream the whole per-core encoder slice into SBUF in
            # compute order; it stays resident (16 KiB/partition fp8).
            eo_t = {}
            for blk in range(nblk):
                for b in range(bpc):
                    t = eopool.tile([P, KC, SBLK], F8, tag=f"eo{blk}_{b}")
                    eo_t[(blk, b)] = t
                    nc.gpsimd.dma_start(
                        t[:], eoT.ap()[:, :, b, blk * SBLK:(blk + 1) * SBLK]
                    )

            for blk in range(nblk):
                sb0 = blk * SBLK
                # all 8 batches' energies accumulate into rows of eps
                eps = epspool.tile([bpc, SBLK], F32, tag="eps")
                for b in range(bpc):
                    eob = eo_t[(blk, b)]
                    sig = sigpool.tile([P, KC, SBLK], BF16, tag="sig")
                    for kc in range(KC):
                        enc = encpool.tile([P, SBLK], F32, tag="enc")
                        for hp in range(KC // 2):
                            for ns in range(nns):
                                nc.tensor.matmul(
                                    enc[:, ns * NMM:(ns + 1) * NMM],
                                    WeT_sb[:, 2 * hp:2 * hp + 2,
                                           kc * P:(kc + 1) * P],
                                    eob[:, 2 * hp:2 * hp + 2,
                                        ns * NMM:(ns + 1) * NMM],
                                    start=(hp == 0),
                                    stop=(hp == KC // 2 - 1),
                                    perf_mode=DR,
                                )
                        nc.scalar.activation(
                            sig[:, kc, :], enc[:], Tanh,
                            scale=0.5 / WE_SCALE, bias=hidb_sb[:, kc, b:b + 1],
                        )
                    # m[p, s] = sum_kc wv[kc*P+p] * sig[p, kc, s]  (DVE)
                    m0 = accpool.tile([P, SBLK], BF16, tag="m0")
                    m1 = accpool.tile([P, SBLK], BF16, tag="m1")
                    m2 = accpool.tile([P, SBLK], BF16, tag="m2")
                    m3 = accpool.tile([P, SBLK], BF16, tag="m3")
                    nc.vector.tensor_scalar_mul(m0[:], sig[:, 0, :], wv_sb[:, 0:1])
                    nc.vector.tensor_scalar_mul(m1[:], sig[:, 1, :], wv_sb[:, 1:2])
                    nc.vector.tensor_scalar_mul(m2[:], sig[:, 2, :], wv_sb[:, 2:3])
                    nc.vector.tensor_scalar_mul(m3[:], sig[:, 3, :], wv_sb[:, 3:4])
                    nc.vector.tensor_add(m0[:], m0[:], m1[:])
                    nc.vector.tensor_add(m2[:], m2[:], m3[:])
                    nc.vector.tensor_add(m0[:], m0[:], m2[:])
                    # E[b, s] = sum_p m0[p, s]: one-hot stationary drops the
                    # row sum into eps row b (other rows accumulate +0).
                    for ns in range(nns):
                        nc.tensor.matmul(
                            eps[:, ns * NMM:(ns + 1) * NMM],
                            oh_sb[:, b, :],
                            m0[:, ns * NMM:(ns + 1) * NMM],
                            start=(b == 0), stop=(b == bpc - 1),
                        )
                # softmax partials: exp reads the PSUM tile directly.
                # |0.5*E| <= 0.5*sum|Wv| ~ 5.7 so exp cannot overflow.
                nc.scalar.activation(
                    pexp[:, sb0:sb0 + SBLK], eps[:, :],
                    Exp, scale=0.5, accum_out=sloc[:, blk:blk + 1],
                )

            # attn = pexp / rowsum(pexp)
            stot = wpool.tile([bpc, 1], F32, tag="stot")
            nc.vector.tensor_reduce(
                out=stot[:], in_=sloc[:],
                axis=mybir.AxisListType.X, op=Add,
            )
            rec = wpool.tile([bpc, 1], F32, tag="rec")
            nc.vector.reciprocal(rec[:], stot[:])
            for blk in range(nblk):
                sb0 = blk * SBLK
                nc.vector.tensor_scalar_mul(
                    pexp[:, sb0:sb0 + SBLK], pexp[:, sb0:sb0 + SBLK],
                    rec[:],
                )
                nc.sync.dma_start(
                    out.ap()[:, sb0:sb0 + SBLK], pexp[:, sb0:sb0 + SBLK]
                )

    nc.compile()
    _BUILD_CACHE[key] = nc
    return nc


def make_in_maps(hidden, encoder_output, We, be, Wh, bh, Wv):
    """Host-side sharding/layout prep. Returns per-core input dicts."""
    import ml_dtypes
    eo8 = np.asarray(encoder_output, dtype=np.float32).astype(
        ml_dtypes.float8_e4m3fn
    )  # [S, B, H]
    hidden = np.asarray(hidden, dtype=np.float32)
    WeT = np.ascontiguousarray(
        (np.asarray(We, np.float32).T * WE_SCALE)
        .reshape(KC, P, H).transpose(1, 0, 2)
    ).astype(ml_dtypes.float8_e4m3fn)  # [P, KC(hc), H(k)]

    # hidb = 0.5 * (hidden @ Wh.T + bh + be), laid out [P, KC, B]
    hid_all = 0.5 * (
        hidden @ np.asarray(Wh, np.float32).T
        + np.asarray(bh, np.float32) + np.asarray(be, np.float32)
    )  # [B_TOT, H]
    wv = np.asarray(Wv, np.float32).reshape(-1)           # [H]
    wvp = np.ascontiguousarray(wv.reshape(KC, P).T)        # [P, KC]
    onehot = np.zeros((P, BPC, BPC), np.float32)
    for b in range(BPC):
        onehot[:, b, b] = 1.0
    onehot = onehot.astype(ml_dtypes.bfloat16)

    in_maps = []
    for c in range(N_CORES):
        b0 = c * BPC
        # [P, KC, BPC, S]: element (p, hc, b, s) = eo[s, b0+b, hc*128+p]
        eoT_c = np.ascontiguousarray(
            eo8[:, b0:b0 + BPC, :].transpose(2, 1, 0)
            .reshape(KC, P, BPC, S_TOT).transpose(1, 0, 2, 3)
        )
        hidb_c = np.ascontiguousarray(
            hid_all[b0:b0 + BPC].T.reshape(KC, P, BPC).transpose(1, 0, 2)
        )  # [P, KC, BPC]
        in_maps.append({
            "eoT": eoT_c,
            "WeT": WeT,
            "hidb": hidb_c,
            "wvp": wvp,
            "onehot": onehot,
        })
    return in_maps


def kernel(hidden, encoder_output, each_size=None, We=None, be=None,
           Wh=None, bh=None, Wv=None, bv=None):
    global LAST_RESULTS
    nc = _build()
    in_maps = make_in_maps(hidden, encoder_output, We, be, Wh, bh, Wv)
    res = run_bass_kernel_spmd(
        nc, in_maps, list(range(N_CORES)),
        trace=bool(os.environ.get("BASS_TRACE")),
    )
    LAST_RESULTS = res
    attn = np.concatenate([res.results[c]["out"] for c in range(N_CORES)], axis=0)
    return np.ascontiguousarray(attn.reshape(B_TOT, 1, S_TOT).astype(np.float32))
